# revision 35
# baseline (speedup 1.0000x reference)
"""Trainium2 Bass kernel: PreActBlock with DoReFa 4-bit quantization (sync-BN).

  out = conv3x3(q(relu(BN1(conv3x3(q(relu(BN0(x))), qw(w0))))), qw(w1)) + x

Design (8 cores, data-parallel over batch 16 -> 2 images/core):
 - Quantized activations are integers 0..15 and quantized weights odd integers
   -15..15 (x scale).  Both are exact in fp8e4 (e4m3) and the PE accumulates
   in fp32, so every conv is computed EXACTLY as integer sums (|S| < 2^20).
 - fp8 DoubleRow matmuls: contraction K=256 per instruction via the
   [P, 2, ...] interleaved layout (2x PE throughput).
 - BN batch stats are all-reduced across the 8 cores (sync-BN semantics).
 - Act layout [P, row, ki, 64]: ki innermost (stride 64 so DoubleRow APs are
   legal) keeps sub-tile dependency intervals row-tight, so conv matmuls can
   start as soon as the first quantized rows land instead of waiting for the
   whole quantize phase.
 - x and the intermediate conv0 output S live entirely in SBUF (no DRAM
   spill/reload), so quant1 -> conv1 starts right after the second AllReduce.
 - Quantize is a 3-engine pipeline: stage 1 computes int8(a*x+b) on scalar
   (c=0, Relu fused) and vector (c=1) in parallel; stage 2 on gpsimd clips to
   [0,15] and writes fp8 act codes.  The int8 RNE cast does the rounding.
 - Stats0 splits img0 (scalar accum passes, scratch written into the
   then-dead S area) from img1 (vector bn_stats) so the first AllReduce
   triggers right after the x DMA lands; x is loaded over 4 DMA queues.
 - Both weight DMAs are triggered at the very top of the program (gpsimd
   queue) and wquant1 runs during the AR0 window / early conv0, so conv1
   starts immediately after AllReduce-1 + coeffs.
 - Cross-partition max uses gpsimd.partition_all_reduce; a dummy call at
   t~0 preloads its ext-isa library off the critical path.
"""
import os
import sys

sys.path.insert(0, "/opt/trn_rl_repo")

import numpy as np

import concourse.bacc as bacc
import concourse.bass as bass
import concourse.bass_isa as bass_isa
import concourse.mybir as mybir
from concourse import tile
from concourse import bass_utils

F32 = mybir.dt.float32
FP8 = mybir.dt.float8e4
I8 = mybir.dt.int8
AX = mybir.AxisListType
OP = mybir.AluOpType
AF = mybir.ActivationFunctionType
PM = mybir.MatmulPerfMode
RO = bass_isa.ReduceOp

P = 128
N_CORES = 8
IMG = 2              # images per core
H = 56
ROWS = 116           # 2 images x (1 pad + 56 + 1 pad) rows
CW = 64              # padded column stride of act rows (ki stride, %16 == 0)
CNT = 50176.0        # global BN count: 16 * 56 * 56
EPS = 1e-5

# per-image 9-row output windows (junk boundary rows 57/58 never computed)
WINDOWS = ([(1 + 9 * k, 9) for k in range(6)] + [(55, 2)] +
           [(59 + 9 * k, 9) for k in range(6)] + [(113, 2)])
GROUPS = [WINDOWS[0:2], WINDOWS[2:6], WINDOWS[6:10], WINDOWS[10:14]]
# tap order: full-width tap (dy=0,dx=1) first so start=True covers all columns
TAPS = [(0, 1), (0, 0), (0, 2), (1, 0), (1, 1), (1, 2), (2, 0), (2, 1), (2, 2)]


def _runs(r0, nr):
    """Interior row-runs of a window: (logical_row, nrows, img, h0)."""
    out = []
    for lo, hi, img, base in ((1, 56, 0, 1), (59, 114, 1, 59)):
        a, b = max(r0, lo), min(r0 + nr - 1, hi)
        if a <= b:
            out.append((a, b - a + 1, img, a - base))
    return out


def build():
    nc = bacc.Bacc("TRN2", target_bir_lowering=False, debug=False,
                   enable_asserts=False, num_devices=N_CORES)

    x_d = nc.dram_tensor("x", [IMG, 256, H, H], F32, kind="ExternalInput")
    # host-permuted to [i, kh, kw, o]
    w_d = [nc.dram_tensor("conv0_w", [256, 3, 3, 256], F32, kind="ExternalInput"),
           nc.dram_tensor("conv1_w", [256, 3, 3, 256], F32, kind="ExternalInput")]
    g_d = [nc.dram_tensor("bn0_gamma", [256], F32, kind="ExternalInput"),
           nc.dram_tensor("bn1_gamma", [256], F32, kind="ExternalInput")]
    b_d = [nc.dram_tensor("bn0_beta", [256], F32, kind="ExternalInput"),
           nc.dram_tensor("bn1_beta", [256], F32, kind="ExternalInput")]
    out_d = nc.dram_tensor("out", [IMG, 256, H, H], F32, kind="ExternalOutput")

    xv = x_d.ap().rearrange("n c h w -> c n h w")       # [256, 2, 56, 56]
    ov = out_d.ap().rearrange("n c h w -> c n h w")

    with tile.TileContext(nc) as tc:
        with tc.tile_pool(name="act", bufs=1) as actp, \
             tc.tile_pool(name="wtp", bufs=1) as wtp, \
             tc.tile_pool(name="wq", bufs=4) as wqp, \
             tc.tile_pool(name="qt", bufs=3) as qtp, \
             tc.tile_pool(name="run", bufs=6) as runp, \
             tc.tile_pool(name="st", bufs=1) as stp, \
             tc.tile_pool(name="ps", bufs=8, space="PSUM") as psp, \
             tc.tile_pool(name="dram", bufs=1, space="DRAM") as drp:

            # ---------- static tiles ----------
            # act layout: [P, row, ki, CW]; image cols 0..55, cols 56..63 are
            # never-read filler that pads the ki stride to 64.
            act0 = actp.tile([P, ROWS, 2, CW], FP8, name="act0")
            act1 = actp.tile([P, ROWS, 2, CW], FP8, name="act1")
            actv = [act0.rearrange("p r k c -> p k r c"),
                    act1.rearrange("p r k c -> p k r c")]
            # quantized weight codes, [ci_lo, tap, ki, co] fp8
            wT = [wtp.tile([P, 9, 2, 256], FP8, name=f"w{v}T") for v in range(2)]
            # full x and conv0-integer-output S, SBUF-resident per co half
            x_sb = [actp.tile([P, IMG, H, H], F32, name=f"x_sb_{c}")
                    for c in range(2)]
            s_sb = [actp.tile([P, IMG, H, H], F32, name=f"s_sb_{c}")
                    for c in range(2)]
            x_fl = [t.rearrange("p i h w -> p (i h w)") for t in x_sb]
            s_fl = [t.rearrange("p i h w -> p (i h w)") for t in s_sb]
            ar_in = [drp.tile([P, 4], F32, name=f"ar_in_{i}") for i in range(2)]
            ar_out = [drp.tile([P, 4], F32, name=f"ar_out_{i}") for i in range(2)]

            # stats / small vectors
            xbn = [stp.tile([P, 8, 6], F32, name=f"xbn_{c}") for c in range(2)]
            gsx = [stp.tile([P, 2], F32, name=f"gsx_{c}") for c in range(2)]
            sqx = [stp.tile([P, 2], F32, name=f"sqx_{c}") for c in range(2)]
            sumS = [stp.tile([P, 16], F32, name=f"sumS_{c}") for c in range(2)]
            sqS = [stp.tile([P, 16], F32, name=f"sqS_{c}") for c in range(2)]
            gvec = [stp.tile([P, 2], F32, name=f"g{v}") for v in range(2)]
            bvec = [stp.tile([P, 2], F32, name=f"b{v}") for v in range(2)]
            avec = [stp.tile([P, 2], F32, name=f"a{v}") for v in range(2)]
            bbvec = [stp.tile([P, 2], F32, name=f"bb{v}") for v in range(2)]
            svec = [stp.tile([P, 1], F32, name=f"scale_{v}") for v in range(2)]
            pk = [stp.tile([P, 4], F32, name=f"pk_{i}") for i in range(2)]
            gpk = [stp.tile([P, 4], F32, name=f"gpk_{i}") for i in range(2)]

            def vtile(name, w=1):
                return stp.tile([P, w], F32, name=name, tag="vtmp", bufs=8,
                                padded_shape=[P, 4])

            # ---------- weight DMA prefetch (gpsimd queue) ----------
            wnat = {}

            def wquant_dma(v):
                wv = w_d[v].ap().rearrange("i kh kw o -> i (kh kw) o")
                for ki in range(2):
                    for hh in range(2):  # tap halves: 0 -> taps 0..3, 1 -> 4..8
                        t0, t1 = (0, 4) if hh == 0 else (4, 9)
                        wn = wqp.tile([P, t1 - t0, 256], F32,
                                      name=f"wn{v}{ki}{hh}", tag=f"wnat{v}",
                                      bufs=4, padded_shape=[P, 5, 256])
                        nc.gpsimd.dma_start(
                            wn[:], wv[ki * P:(ki + 1) * P, t0:t1, :])
                        wnat[(v, ki, hh)] = wn

            # ---------- x load: 8 chunks over 3 DMA queues ----------
            # gpsimd queue carries 2 chunks FIRST (before the weight loads).
            # img1 blk0 rides the gpsimd queue (lands first, and the vector
            # bn_stats FIFO consumes img1 chunks in this same order)
            qmap = [nc.sync.dma_start, nc.scalar.dma_start,
                    nc.sync.dma_start, nc.scalar.dma_start,
                    nc.gpsimd.dma_start, nc.gpsimd.dma_start,
                    nc.sync.dma_start, nc.scalar.dma_start]
            qi = 0
            for img in range(IMG):
                for blk in range(2):
                    for c in range(2):
                        h0 = blk * 28
                        qmap[qi](x_sb[c][:, img, h0:h0 + 28, :],
                                 xv[c * P:(c + 1) * P, img, h0:h0 + 28, :])
                        qi += 1

            # ---------- act pad-row zeroing ----------
            with nc.named_scope("memset"):
                for t in (act0, act1):
                    for r in (0, 57, 58, 115):
                        nc.gpsimd.memset(t[:, r, :, 0:H], 0.0)

            # preload the gpsimd ext-isa lib (partition_all_reduce) off the
            # critical path: the ~7us MODIFY_POOL_CONFIG runs here, after all
            # gpsimd DMA triggers have fired and long before the first use.
            pdum = stp.tile([P, 1], F32, name="pdum")
            pdum2 = stp.tile([P, 1], F32, name="pdum2")
            nc.gpsimd.memset(pdum[:], 0.0)
            nc.gpsimd.partition_all_reduce(pdum2[:], pdum[:], P, RO.max)

            # ---------- BN0 stats: img0 on scalar, img1 on vector ----------
            with nc.named_scope("stats0"):
                for img in range(IMG):
                    for blk in range(2):
                        for c in range(2):
                            h0 = blk * 28
                            ch = x_sb[c][:, img, h0:h0 + 28, :]
                            if img == 0:
                                # scratch into the (still dead) S region
                                sdst = s_sb[c][:, img, h0:h0 + 28, :]
                                nc.scalar.activation(
                                    sdst, ch, AF.Square,
                                    accum_out=sqx[c][:, blk:blk + 1])
                                nc.scalar.activation(
                                    sdst, ch, AF.Identity,
                                    accum_out=gsx[c][:, blk:blk + 1])
                            else:
                                fl = x_fl[c][:, (img * H + h0) * H:
                                             (img * H + h0 + 28) * H]
                                k = blk * 4
                                for cc in range(4):
                                    nc.vector.bn_stats(
                                        xbn[c][:, k + cc, :],
                                        fl[:, cc * 392:(cc + 1) * 392])
                for c in range(2):
                    mv = stp.tile([P, 2], F32, name=f"mv0_{c}")
                    nc.vector.bn_aggr(mv[:], xbn[c][:, 0:8, :])
                    # vector part is img1 (n_v = 3136); scalar accums are img0
                    m2 = vtile(f"xm2_{c}")
                    nc.vector.tensor_mul(m2[:], mv[:, 0:1], mv[:, 0:1])
                    vp = vtile(f"xvp_{c}")
                    nc.vector.tensor_add(vp[:], mv[:, 1:2], m2[:])
                    ss = vtile(f"ss_{c}")
                    nc.vector.tensor_add(ss[:], gsx[c][:, 0:1], gsx[c][:, 1:2])
                    qs = vtile(f"qs_{c}")
                    nc.vector.tensor_add(qs[:], sqx[c][:, 0:1], sqx[c][:, 1:2])
                    nc.vector.scalar_tensor_tensor(
                        pk[0][:, c:c + 1], mv[:, 0:1], 3136.0, ss[:],
                        OP.mult, OP.add)
                    nc.vector.scalar_tensor_tensor(
                        pk[0][:, 2 + c:3 + c], vp[:], 3136.0, qs[:],
                        OP.mult, OP.add)
                nc.sync.dma_start(ar_in[0][:], pk[0][:])
                nc.gpsimd.collective_compute(
                    "AllReduce", OP.add, replica_groups=[list(range(N_CORES))],
                    ins=[ar_in[0].opt()], outs=[ar_out[0].opt()])
                nc.sync.dma_start(gpk[0][:], ar_out[0][:])

            # Weight + BN-param DMAs all deferred past the AR0 doorbell (it
            # does not block the gpsimd queue): the x load gets the early HBM
            # window exclusively, pulling the AR0 trigger earlier.  w0 lands
            # ~10us after the doorbell, well before tanh0's slot.
            wquant_dma(0)
            for v in range(2):
                for c in range(2):
                    nc.gpsimd.dma_start(gvec[v][:, c:c + 1],
                                        g_d[v].ap()[c * P:(c + 1) * P])
                    nc.gpsimd.dma_start(bvec[v][:, c:c + 1],
                                        b_d[v].ap()[c * P:(c + 1) * P])
            wquant_dma(1)

            # ---------- weight quantization (tanh phase / codes phase) -------
            def wquant_tanh(v):
                for ki in range(2):
                    for hh in range(2):
                        wf = wnat[(v, ki, hh)].rearrange("p a b -> p (a b)")
                        nc.scalar.activation(wf[:], wf[:], AF.Tanh)

            mvec = [None, None]

            def wquant_codes_reduce(v):
                mxp = stp.tile([P, 4], F32, name=f"mxp_{v}")
                for i, (ki, hh) in enumerate(((0, 0), (0, 1), (1, 0), (1, 1))):
                    wf = wnat[(v, ki, hh)].rearrange("p a b -> p (a b)")
                    nc.vector.tensor_reduce(
                        mxp[:, i:i + 1], wf[:], AX.X, OP.max,
                        apply_absolute_value=True)
                mx1 = vtile(f"mx1_{v}")
                nc.vector.tensor_reduce(mx1[:], mxp[:], AX.X, OP.max,
                                        apply_absolute_value=True)
                mvec[v] = stp.tile([P, 1], F32, name=f"mvec_{v}")
                nc.gpsimd.partition_all_reduce(mvec[v][:], mx1[:], P, RO.max)

            def wquant_codes_tail(v):
                # svec = M/225 (psum scale); sc = 7.5/M for codes
                nc.vector.tensor_scalar(svec[v][:], mvec[v][:], 1.0 / 225.0,
                                        None, OP.mult)
                r = vtile(f"rin_{v}")
                nc.vector.reciprocal(r[:], mvec[v][:])
                for i in range(2):  # Newton: r = r*(2 - M*r)
                    t1_ = vtile(f"rn1_{v}{i}")
                    nc.vector.tensor_mul(t1_[:], mvec[v][:], r[:])
                    t2_ = vtile(f"rn2_{v}{i}")
                    nc.vector.tensor_scalar(t2_[:], t1_[:], -1.0, 2.0,
                                            OP.mult, OP.add)
                    rn = vtile(f"rn3_{v}{i}")
                    nc.vector.tensor_mul(rn[:], r[:], t2_[:])
                    r = rn
                sc = vtile(f"sc_{v}")
                nc.vector.tensor_scalar(sc[:], r[:], 7.5, None, OP.mult)
                for i, (ki, hh) in enumerate(((0, 0), (0, 1), (1, 0), (1, 1))):
                    wn = wnat[(v, ki, hh)]
                    sh = list(wn.shape)
                    wf = wn.rearrange("p a b -> p (a b)")
                    # z = tanh*sc + 7.5 in-place
                    nc.vector.tensor_scalar(wf[:], wf[:], sc[:], 7.5,
                                            OP.mult, OP.add)
                    ri = wqp.tile(sh, I8, name=f"ri{v}{ki}{hh}", tag="wr",
                                  bufs=2, padded_shape=[P, 5, 256])
                    rf = ri.rearrange("p a b -> p (a b)")
                    nc.vector.tensor_scalar(rf[:], wf[:], 0.0, 15.0,
                                            OP.max, OP.min)
                    t0 = 0 if hh == 0 else 4
                    dst = wT[v][:, t0:t0 + sh[1], ki, :]
                    nc.vector.tensor_scalar(dst, ri[:], 2.0, -15.0,
                                            OP.mult, OP.add)

            with nc.named_scope("wquant0"):
                wquant_tanh(0)
                wquant_codes_reduce(0)
                wquant_codes_tail(0)

            # ---------- BN coefficients: z = a*S + b on [P, 2] ----------
            def bn_coeffs(v, scale):
                # me = [mean_c0, mean_c1, ex2_c0, ex2_c1]
                me = vtile(f"me{v}", 4)
                nc.vector.tensor_scalar(me[:], gpk[v][:, 0:4], 1.0 / CNT,
                                        None, OP.mult)
                mean, ex2 = me[:, 0:2], me[:, 2:4]
                if scale is not None:
                    s2 = vtile(f"s2{v}")
                    nc.vector.tensor_mul(s2[:], scale[:], scale[:])
                    nc.vector.tensor_scalar(ex2, ex2, s2[:], None, OP.mult)
                    nc.vector.tensor_scalar(mean, mean, scale[:], None, OP.mult)
                m2 = vtile(f"m2{v}", 2)
                nc.vector.tensor_mul(m2[:], mean, mean)
                vpe = vtile(f"vp{v}", 2)
                nc.vector.tensor_sub(vpe[:], ex2, m2[:])
                nc.vector.tensor_scalar(vpe[:], vpe[:], EPS, None, OP.add)
                # rsqrt: reciprocal + sqrt + 1 Newton refinement
                rr = vtile(f"rr{v}", 2)
                nc.vector.reciprocal(rr[:], vpe[:])
                y = vtile(f"y{v}", 2)
                nc.scalar.activation(y[:], rr[:], AF.Sqrt)
                y2 = vtile(f"y2{v}", 2)
                nc.vector.tensor_mul(y2[:], y[:], y[:])
                t2 = vtile(f"t2{v}", 2)
                nc.vector.tensor_mul(t2[:], vpe[:], y2[:])
                h = vtile(f"h{v}", 2)
                nc.vector.tensor_scalar(h[:], t2[:], -0.5, 1.5, OP.mult, OP.add)
                grs = vtile(f"gr{v}", 2)
                nc.vector.tensor_mul(grs[:], y[:], h[:])
                nc.vector.tensor_mul(grs[:], gvec[v][:], grs[:])
                if scale is not None:
                    # quantize input is the raw integer S: a = 15*scale*g*rs
                    ga = vtile(f"ga{v}", 2)
                    nc.vector.tensor_scalar(ga[:], grs[:], scale[:], None,
                                            OP.mult)
                    nc.vector.tensor_scalar(avec[v][:], ga[:], 15.0, None,
                                            OP.mult)
                else:
                    nc.vector.tensor_scalar(avec[v][:], grs[:], 15.0, None,
                                            OP.mult)
                mg = vtile(f"mg{v}", 2)
                nc.vector.tensor_mul(mg[:], mean, grs[:])
                nc.vector.tensor_scalar(mg[:], mg[:], 15.0, None, OP.mult)
                nc.vector.scalar_tensor_tensor(
                    bbvec[v][:], bvec[v][:], 15.0, mg[:],
                    OP.mult, OP.subtract)

            with nc.named_scope("coeffs0"):
                bn_coeffs(0, None)

            # ---------- quantize pipeline ----------
            # stage1: int8(relu(a*x+b)) on scalar (RNE cast rounds)
            # stage2: vector min(.,15) -> fp8 act codes
            def quantize_block(src_t, act_t, c, img, h0, nr, v, names):
                lr = img * 58 + 1 + h0   # logical row
                src_ch = src_t[:, img, h0:h0 + nr, :]
                u = qtp.tile([P, nr, H], I8, name=names + "u", tag="qu",
                             bufs=4, padded_shape=[P, 28, H])
                nc.scalar.activation(u[:], src_ch, AF.Relu,
                                     bias=bbvec[v][:, c:c + 1],
                                     scale=avec[v][:, c:c + 1])
                nc.vector.tensor_scalar(act_t[:, lr:lr + nr, c, 0:H], u[:],
                                        15.0, None, OP.min)

            def quant_phase(src, act_t, v, tag):
                # first block split in half so the first conv window's rows
                # land ~1.7us sooner after the AllReduce
                for c in range(2):
                    quantize_block(src[c], act_t, c, 0, 0, 14, v,
                                   f"{tag}_{c}00a")
                for c in range(2):
                    quantize_block(src[c], act_t, c, 0, 14, 14, v,
                                   f"{tag}_{c}00b")
                for img in range(IMG):
                    for blk in range(2):
                        if img == 0 and blk == 0:
                            continue
                        for c in range(2):
                            quantize_block(src[c], act_t, c, img, blk * 28,
                                           28, v, f"{tag}_{c}{img}{blk}")

            with nc.named_scope("quant0"):
                quant_phase(x_sb, act0, 0, "q0")

            with nc.named_scope("wquant1"):
                wquant_tanh(1)
                wquant_codes_reduce(1)

            # ---------- conv (shared), fp8 DoubleRow, K=256 per matmul ----------
            def conv(v, epilogue):
                av = actv[v]
                for gi, grp in enumerate(GROUPS):
                    for co in range(2):
                        psums = []
                        for wi, (r0, nr) in enumerate(grp):
                            ps = psp.tile([P, nr, H], F32,
                                          name=f"ps{v}_{gi}_{co}_{wi}",
                                          tag="psw", padded_shape=[P, 9, H])
                            psums.append(ps)
                        for ti, (dy, dx) in enumerate(TAPS):
                            tap = dy * 3 + dx
                            wlo, whi = max(0, 1 - dx), min(H, H + 1 - dx)
                            jlo = max(0, dx - 1)
                            lhsT = wT[v][:, tap, :, co * P:(co + 1) * P]
                            first = ti == 0
                            last = ti == 8
                            for wi, (r0, nr) in enumerate(grp):
                                rows = slice(r0 + dy - 1, r0 + dy - 1 + nr)
                                rhs = av[:, :, rows, jlo:jlo + whi - wlo]
                                if dx == 1:
                                    out = psums[wi][:, :, :]
                                else:
                                    out = psums[wi][:, :, wlo:whi]
                                nc.tensor.matmul(out, lhsT, rhs,
                                                 start=first, stop=last,
                                                 perf_mode=PM.DoubleRow)
                        for wi, (r0, nr) in enumerate(grp):
                            epilogue(co, r0, nr, psums[wi])

            # ---------- conv0 epilogue: S -> SBUF + interior sums ----------
            slot_idx = [0, 0]

            def epi0(co, r0, nr, ps):
                psf = ps.rearrange("p r c -> p (r c)")
                for (rl, n, img, h0) in _runs(r0, nr):
                    sl = psf[:, (rl - r0) * H:(rl - r0 + n) * H]
                    dst = s_fl[co][:, (img * H + h0) * H:(img * H + h0 + n) * H]
                    k = slot_idx[co]
                    slot_idx[co] += 1
                    nc.scalar.activation(dst, sl, AF.Identity,
                                         accum_out=sumS[co][:, k:k + 1])
                    sq = runp.tile([P, n * H], F32, name=f"sq_{co}_{rl}",
                                   tag="sq", bufs=2, padded_shape=[P, 9 * H])
                    nc.vector.scalar_tensor_tensor(
                        sq[:], dst, 0.0, dst, OP.bypass, OP.mult,
                        accum_out=sqS[co][:, k:k + 1])

            with nc.named_scope("conv0"):
                conv(0, epi0)

            # ---------- BN1 stats + AR ----------
            with nc.named_scope("stats1"):
                for c in range(2):
                    ns = slot_idx[c]
                    nc.vector.tensor_reduce(pk[1][:, c:c + 1],
                                            sumS[c][:, 0:ns], AX.X, OP.add)
                    nc.vector.tensor_reduce(pk[1][:, 2 + c:3 + c],
                                            sqS[c][:, 0:ns], AX.X, OP.add)
                nc.sync.dma_start(ar_in[1][:], pk[1][:])
                nc.gpsimd.collective_compute(
                    "AllReduce", OP.add, replica_groups=[list(range(N_CORES))],
                    ins=[ar_in[1].opt()], outs=[ar_out[1].opt()])
                nc.sync.dma_start(gpk[1][:], ar_out[1][:])

            with nc.named_scope("wquant1t"):
                wquant_codes_tail(1)

            with nc.named_scope("coeffs1"):
                bn_coeffs(1, svec[0])

            # ---------- quantize1: S -> act1 codes ----------
            with nc.named_scope("quant1"):
                quant_phase(s_sb, act1, 1, "q1")

            # ---------- conv1 + residual epilogue ----------
            def epi1(co, r0, nr, ps):
                psf = ps.rearrange("p r c -> p (r c)")
                for (rl, n, img, h0) in _runs(r0, nr):
                    sl = psf[:, (rl - r0) * H:(rl - r0 + n) * H]
                    xt = x_fl[co][:, (img * H + h0) * H:(img * H + h0 + n) * H]
                    ot = runp.tile([P, n * H], F32, name=f"o_{co}_{rl}",
                                   tag="orun", bufs=5, padded_shape=[P, 9 * H])
                    nc.vector.scalar_tensor_tensor(
                        ot[:], sl, svec[1][:], xt, OP.mult, OP.add)
                    nc.sync.dma_start(
                        ov[co * P:(co + 1) * P, img, h0:h0 + n, :],
                        ot.rearrange("p (a b) -> p a b", b=H)[:])

            with nc.named_scope("conv1"):
                conv(1, epi1)

    nc.compile()
    return nc


def _install_ntff_hook():
    """Provide antenv.axon_hooks (absent in this image) via ctypes so that
    run_bass_kernel_spmd(trace=True) can capture NTFF profiles."""
    try:
        from antenv.axon_hooks import get_axon_ntff_profile_hook  # noqa: F401
        return
    except ImportError:
        pass
    import contextlib
    import ctypes
    import types

    so_path = "/opt/axon/libaxon_pjrt.so"
    if not os.path.exists(so_path):
        return
    lib = ctypes.CDLL(so_path)
    if not hasattr(lib, "axon_start_nrt_profile"):
        return
    lib.axon_start_nrt_profile.argtypes = [ctypes.POINTER(ctypes.c_int64),
                                           ctypes.c_size_t]
    lib.axon_start_nrt_profile.restype = ctypes.c_int64
    lib.axon_stop_nrt_profile.argtypes = [ctypes.c_char_p]
    lib.axon_stop_nrt_profile.restype = ctypes.c_int64

    @contextlib.contextmanager
    def _hook(output_dir, device_ids):
        import jax
        jax.devices()
        if device_ids:
            ids = (ctypes.c_int64 * len(device_ids))(*device_ids)
            rc = lib.axon_start_nrt_profile(ids, len(device_ids))
        else:
            rc = lib.axon_start_nrt_profile(None, 0)
        if rc != 0:
            raise RuntimeError(f"axon_start_nrt_profile rc={rc}")
        try:
            yield
        finally:
            n = lib.axon_stop_nrt_profile(str(output_dir).encode())
            print(f"ntff profile: {n} file(s) written to {output_dir}")

    hook_holder = [_hook]
    mod = types.ModuleType("antenv.axon_hooks")
    mod.get_axon_ntff_profile_hook = lambda: hook_holder[0]
    mod.set_axon_ntff_profile_hook = lambda h: hook_holder.__setitem__(0, h)
    import antenv
    sys.modules["antenv.axon_hooks"] = mod
    antenv.axon_hooks = mod


_NC = None


def _get_nc():
    global _NC
    if _NC is None:
        _NC = build()
    return _NC


LAST_RESULTS = None


def kernel(x, bn0_gamma, bn0_beta, conv0_w, bn1_gamma, bn1_beta, conv1_w):
    global LAST_RESULTS
    nc = _get_nc()
    shared = {
        # permute OIHW -> [i, kh, kw, o] so on-chip weight access is contiguous
        "conv0_w": np.ascontiguousarray(
            np.asarray(conv0_w, np.float32).transpose(1, 2, 3, 0)),
        "conv1_w": np.ascontiguousarray(
            np.asarray(conv1_w, np.float32).transpose(1, 2, 3, 0)),
        "bn0_gamma": np.ascontiguousarray(bn0_gamma, np.float32),
        "bn0_beta": np.ascontiguousarray(bn0_beta, np.float32),
        "bn1_gamma": np.ascontiguousarray(bn1_gamma, np.float32),
        "bn1_beta": np.ascontiguousarray(bn1_beta, np.float32),
    }
    x = np.ascontiguousarray(x, np.float32)
    in_maps = [{"x": x[2 * c:2 * c + 2], **shared} for c in range(N_CORES)]
    trace = bool(int(os.environ.get("KERNEL_TRACE", "0")))
    if trace:
        _install_ntff_hook()
    res = bass_utils.run_bass_kernel_spmd(
        nc, in_maps, core_ids=list(range(N_CORES)), trace=trace)
    LAST_RESULTS = res
    return np.concatenate([res.results[c]["out"] for c in range(N_CORES)], axis=0)
